# revision 13
# baseline (speedup 1.0000x reference)
"""Trainium2 Bass kernel for nn_AdjacencyMatrixLayer.

Computes, per batch sample b (coordinates x in R^{N x 3}):
    d_ij  = |x_i|^2 - 2 x_i.x_j + |x_j|^2
    A     = exp(-d / sigma^2)
    A     = softmax(A, axis=2) * mask
    out   = A / (sum_j A + 1e-20)

Device-side restructuring:
  * One K=22 bf16 matmul produces y = -d/sigma^2 + C*(v_i*v_j - 1):
    the first 20 rows are the hi/lo-split augmented coordinates (exact
    to ~2^-18); the last 2 rows fold the padding mask into the PE
    (C=144, v = 0/1 valid bits), so masked entries get y - 144 and the
    mask tensor is never shipped or multiplied.
  * The double exponential F(y) = exp(exp(y)) is approximated, up to a
    global per-row-cancelling scale K, by a quadratic in p = exp(s*y):
        q = p^2 + a*p + b ~= K * exp(exp(y)),  minimax rel err 5.1e-3
    (cubic variant: q = p^3+a p^2+b p+c, rel err 3.2e-4, one more DVE
    pass).  Masked entries give p = 0 exactly (exp underflow), q = b,
    and contribute b per element to the row sum, so
        sum_valid q = accum(t) + b*L_b
    with L_b the per-sample valid length — a host-provided constant.
    Softmax renormalization makes K and the final 1/sum exact:
        out = q / sum_valid(q)   on the valid [L,L] block
    and the host zero-fills the masked region of the output.
  * Engine placement per [128, 2048] row-block:
      PE : 4 bf16 matmuls (512-col PSUM banks)        ~1.9us
      ACT: p = Exp(scale*psum), fp16 out              ~2.0us
      DVE: t = (p + a) * p, accum -> qs   (stt is 1x on TRN2 HW: 2.27us;
           TENSOR_SCALAR is the only 4x op, and needs one tensor input)
      Pool: qs2 = qs + b*L (tiny) ; DVE: r = 1/qs2 (tiny)
      scale pass out = (t + b) * r: DVE tensor_scalar (4x, 0.75us) for
           19/32 blocks, ACT Identity(t*r + b*r) for 13/32 blocks —
           balances ACT ~90us vs DVE ~90us instead of DVE-bound 98us
      DMA: out row-block, alternating SWDGE (gpsimd) / HWDGE (sync)
  * Measured per-core: span ~113.5us, scalar/vector ~90us each,
    tensor ~61us, dma ~53us.  (baseline 144.6us; rel err 1.04e-2)
"""

import sys

import numpy as np

for _p in ("/opt/trn_rl_repo", "/root/.axon_site/_ro/trn_rl_repo"):
    if _p not in sys.path:
        sys.path.append(_p)

B, N, D = 16, 2048, 3
NCORES = 8
SPC = B // NCORES  # samples per core
P = 128            # SBUF partitions
MMF = 512          # matmul moving free-dim chunk (= 1 PSUM bank of fp32)
NB = SPC * N // P  # row-blocks per core
KAUG = 22          # 20 hi/lo aug rows + 2 mask-fold rows
MASKC = 144.0      # mask fold offset: masked entries get y - 144

MODE = "quad"      # "quad" (2 DVE passes) or "cubic" (3 DVE passes)
POOL_STT = 0       # Pool Q7 firmware lacks stt/ts; keep all stt on DVE
POOL_ACCUM = True  # Pool stt supports accum_out (fallback: extra DVE reduce)
ACT_SCALE = 13     # final-scale passes moved to ACT Copy (out = t*r + b*r)

# minimax fits of (poly in p) / (K * exp(exp(y))) - 1 over y <= 0
QS_S = 0.9943403856229558   # p = exp(QS_S * y)
QS_A = 1.05888673672267     # q = p^2 + QS_A*p + QS_B
QS_B = 1.217950642291432
CU_A = 1.600139700859946    # q = p^3 + CU_A*p^2 + CU_B*p + CU_C, p = exp(y)
CU_B = 3.7300379796011542
CU_C = 3.6840145818755072

_CACHE: dict = {}


def _build(mode):
    import concourse.bacc as bacc
    import concourse.tile as tile
    from concourse import mybir

    f32 = mybir.dt.float32
    f16 = mybir.dt.float16
    bf16 = mybir.dt.bfloat16
    AT = mybir.AluOpType
    nc = bacc.Bacc(None, target_bir_lowering=False, debug=False)

    aug_x = nc.dram_tensor("aug_x", [SPC, KAUG, N], bf16, kind="ExternalInput")
    aug_y = nc.dram_tensor("aug_y", [SPC, KAUG, N], bf16, kind="ExternalInput")
    # per-sample row-sum correction, replicated across partitions:
    # quad: b*L ; cubic: c*L
    cons = nc.dram_tensor("cons", [SPC, P, 1], f32, kind="ExternalInput")
    out = nc.dram_tensor("out", [SPC, N, N], f16, kind="ExternalOutput")

    o_flat = out.rearrange("s n m -> (s n) m")

    with tile.TileContext(nc) as tc:
        with (
            tc.tile_pool(name="consts", bufs=1) as consts,
            tc.tile_pool(name="work", bufs=10) as workp,
            tc.tile_pool(name="ot", bufs=8) as otp,
            tc.tile_pool(name="small", bufs=16) as smallp,
            tc.tile_pool(name="psum", bufs=2, space="PSUM") as psump,
        ):
            augx_t, augy_t, cons_t = [], [], []
            bconst = consts.tile([P, 1], f32, tag="bconst")
            for s in range(SPC):
                axc = [consts.tile([KAUG, MMF], bf16, tag=f"ax{s}_{j}",
                                   name=f"ax{s}_{j}")
                       for j in range(N // MMF)]
                ayc = [consts.tile([KAUG, MMF], bf16, tag=f"ay{s}_{j}",
                                   name=f"ay{s}_{j}")
                       for j in range(N // MMF)]
                for j in range(N // MMF):
                    eng = nc.sync if j % 2 == 0 else nc.gpsimd
                    eng.dma_start(out=axc[j],
                                  in_=aug_x[s][:, j * MMF:(j + 1) * MMF])
                    eng2 = nc.gpsimd if j % 2 == 0 else nc.sync
                    eng2.dma_start(out=ayc[j],
                                   in_=aug_y[s][:, j * MMF:(j + 1) * MMF])
                cn = consts.tile([P, 1], f32, tag=f"cons{s}")
                nc.sync.dma_start(out=cn, in_=cons[s])
                augx_t.append(axc)
                augy_t.append(ayc)
                cons_t.append(cn)

            nc.gpsimd.memset(bconst, QS_B if mode == "quad" else CU_C)

            npool = 0
            for ib in range(NB):
                s = ib // (N // P)
                i0 = (ib % (N // P)) * P

                ps = psump.tile([P, N], f32)
                jx, xo = i0 // MMF, i0 % MMF
                for j in range(N // MMF):
                    nc.tensor.matmul(
                        ps[:, j * MMF:(j + 1) * MMF],
                        augx_t[s][jx][:, xo:xo + P],
                        augy_t[s][j],
                    )

                # Bresenham spread of POOL_STT pool-blocks over NB
                use_pool = ((ib + 1) * POOL_STT // NB) != (ib * POOL_STT // NB)
                npool += use_pool
                stt_eng = nc.gpsimd if use_pool else nc.vector

                p = workp.tile([P, N], f16, tag="p")
                qs = smallp.tile([P, 1], f32, tag="qs")
                if mode == "quad":
                    # p = exp(s*y); t = (p + a)*p ; qs = sum_j t
                    nc.scalar.activation(
                        p, ps, mybir.ActivationFunctionType.Exp, scale=QS_S
                    )
                    t = workp.tile([P, N], f16, tag="t")
                    if use_pool and not POOL_ACCUM:
                        stt_eng.scalar_tensor_tensor(
                            out=t, in0=p, scalar=QS_A, in1=p,
                            op0=AT.add, op1=AT.mult,
                        )
                        # row sum via a cheap DVE 4x pass into a scratch tile
                        tsc = workp.tile([P, N], f16, tag="tsc")
                        nc.vector.tensor_scalar(
                            out=tsc, in0=t, scalar1=1.0, scalar2=None,
                            op0=AT.mult, accum_out=qs,
                        )
                    else:
                        stt_eng.scalar_tensor_tensor(
                            out=t, in0=p, scalar=QS_A, in1=p,
                            op0=AT.add, op1=AT.mult, accum_out=qs,
                        )
                    cfin = QS_B
                else:
                    # p = exp(y); t1 = (p + a)*p ; t = (t1 + b)*p ; qs = sum t
                    nc.scalar.activation(p, ps, mybir.ActivationFunctionType.Exp)
                    t1 = workp.tile([P, N], f16, tag="t1")
                    stt_eng.scalar_tensor_tensor(
                        out=t1, in0=p, scalar=CU_A, in1=p,
                        op0=AT.add, op1=AT.mult,
                    )
                    t = workp.tile([P, N], f16, tag="t")
                    stt_eng.scalar_tensor_tensor(
                        out=t, in0=t1, scalar=CU_B, in1=p,
                        op0=AT.add, op1=AT.mult, accum_out=qs,
                    )
                    cfin = CU_C

                # qs2 = qs + const*L ; r = 1/qs2   (tiny [P,1] ops on DVE)
                qs2 = smallp.tile([P, 1], f32, tag="qs2")
                nc.gpsimd.tensor_tensor(
                    out=qs2, in0=qs, in1=cons_t[s], op=AT.add
                )
                r = smallp.tile([P, 1], f32, tag="r")
                nc.vector.reciprocal(r, qs2)

                # out = (t + cfin) * r
                ot = otp.tile([P, N], f16, tag="ot")
                use_act = ((ib + 1) * ACT_SCALE // NB) != (ib * ACT_SCALE // NB)
                if use_act:
                    # ACT Copy computes t*scale + bias with [P,1] APs
                    br = smallp.tile([P, 1], f32, tag="br")
                    nc.gpsimd.tensor_tensor(
                        out=br, in0=r, in1=bconst, op=AT.mult
                    )
                    nc.scalar.activation(
                        ot, t, mybir.ActivationFunctionType.Identity,
                        scale=r, bias=br,
                    )
                else:
                    nc.vector.tensor_scalar(
                        out=ot, in0=t, scalar1=cfin, scalar2=r,
                        op0=AT.add, op1=AT.mult,
                    )
                out_eng = nc.gpsimd if ib % 2 == 0 else nc.sync
                nc_eng = out_eng
                nc_eng.dma_start(out=o_flat[ib * P:(ib + 1) * P, :], in_=ot)

    nc.compile()
    return nc


def _lengths_from_masks(masks):
    """Per-sample valid lengths; verifies the product-prefix structure."""
    diag = np.einsum('bii->bi', masks)
    valid = (diag > 0.5).astype(np.float32)
    lengths = valid.sum(axis=1).astype(np.int64)
    # prefix check + product check (cheap, exact)
    n = masks.shape[1]
    pref = (np.arange(n)[None, :] < lengths[:, None]).astype(np.float32)
    if not np.array_equal(valid, pref):
        return None
    if not np.array_equal(masks, valid[:, :, None] * valid[:, None, :]):
        return None
    return lengths, valid


def _prepare(coordinates, masks, sigma):
    """Host-side prep: shard over cores, build augmented coordinates."""
    import ml_dtypes

    bf = ml_dtypes.bfloat16
    coords = np.ascontiguousarray(np.asarray(coordinates, dtype=np.float32))
    masks = np.asarray(masks, dtype=np.float32)
    sig = float(np.asarray(sigma, dtype=np.float32).reshape(-1)[0])

    res = _lengths_from_masks(masks)
    assert res is not None, "masks are not product-of-prefix form"
    lengths, valid = res

    norms = np.sum(coords * coords, axis=2, dtype=np.float32)  # [B, N]
    xT = np.swapaxes(coords, 1, 2)                             # [B, 3, N]
    nss = np.float32(-1.0 / (sig * sig))
    aug_x = np.empty((B, 5, N), np.float32)
    aug_x[:, 0:3] = (-2.0 * nss) * xT
    aug_x[:, 3] = nss * norms
    aug_x[:, 4] = nss
    aug_y = np.empty((B, 5, N), np.float32)
    aug_y[:, 0:3] = xT
    aug_y[:, 3] = 1.0
    aug_y[:, 4] = norms

    # hi/lo bf16 split: v = hi + lo, K=5 fp32 -> K=20 bf16 contraction
    xh = aug_x.astype(bf)
    xl = (aug_x - xh.astype(np.float32)).astype(bf)
    yh = aug_y.astype(bf)
    yl = (aug_y - yh.astype(np.float32)).astype(bf)
    # mask fold rows: C*v_i*v_j - C  (exact in bf16: C=144, v in {0,1})
    C = np.float32(MASKC)
    mx = np.stack([C * valid, np.full_like(valid, C)], axis=1).astype(bf)
    my = np.stack([valid, np.full_like(valid, -1.0)], axis=1).astype(bf)
    augx22 = np.concatenate([xh, xl, xh, xl, mx], axis=1)  # [B, 22, N]
    augy22 = np.concatenate([yh, yh, yl, yl, my], axis=1)

    ccoef = QS_B if MODE == "quad" else CU_C
    consv = (np.float32(ccoef) * lengths.astype(np.float32))  # [B]
    cons = np.broadcast_to(consv[:, None, None], (B, P, 1)).astype(np.float32)

    in_maps = []
    for c in range(NCORES):
        lo, hi = c * SPC, (c + 1) * SPC
        in_maps.append({
            "aug_x": np.ascontiguousarray(augx22[lo:hi]),
            "aug_y": np.ascontiguousarray(augy22[lo:hi]),
            "cons": np.ascontiguousarray(cons[lo:hi]),
        })
    return in_maps, lengths


def _get_nc():
    if "nc" not in _CACHE:
        _CACHE["nc"] = _build(MODE)
    return _CACHE["nc"]


def kernel(coordinates, masks, sigma):
    import time

    from concourse.bass_utils import run_bass_kernel_spmd

    in_maps, lengths = _prepare(coordinates, masks, sigma)
    # the shared trn2 device occasionally reports a transient
    # NRT_EXEC_UNIT_UNRECOVERABLE; it clears on its own within ~a minute
    for attempt in range(4):
        try:
            res = run_bass_kernel_spmd(
                _get_nc(), in_maps, core_ids=list(range(NCORES))
            )
            break
        except Exception:  # noqa: BLE001 - retry transient device errors
            if attempt == 3:
                raise
            time.sleep(20 * (attempt + 1))

    full = np.zeros((B, N, N), np.float32)
    for b in range(B):
        c, s = b // SPC, b % SPC
        L = int(lengths[b])
        full[b, :L, :L] = res.results[c]["out"][s, :L, :L].astype(np.float32)
    return full


# revision 14
# speedup vs baseline: 1.0324x; 1.0324x over previous
"""Trainium2 Bass kernel for nn_AdjacencyMatrixLayer.

Computes, per batch sample b (coordinates x in R^{N x 3}):
    d_ij  = |x_i|^2 - 2 x_i.x_j + |x_j|^2
    A     = exp(-d / sigma^2)
    A     = softmax(A, axis=2) * mask
    out   = A / (sum_j A + 1e-20)

Device-side restructuring:
  * One K=22 bf16 matmul produces y = -d/sigma^2 + C*(v_i*v_j - 1):
    the first 20 rows are the hi/lo-split augmented coordinates (exact
    to ~2^-18); the last 2 rows fold the padding mask into the PE
    (C=144, v = 0/1 valid bits), so masked entries get y - 144 and the
    mask tensor is never shipped or multiplied.
  * The double exponential F(y) = exp(exp(y)) is approximated, up to a
    global per-row-cancelling scale K, by a quadratic in p = exp(s*y):
        q = p^2 + a*p + b ~= K * exp(exp(y)),  minimax rel err 5.1e-3
    (cubic variant: q = p^3+a p^2+b p+c, rel err 3.2e-4, one more DVE
    pass).  Masked entries give p = 0 exactly (exp underflow), q = b,
    and contribute b per element to the row sum, so
        sum_valid q = accum(t) + b*L_b
    with L_b the per-sample valid length — a host-provided constant.
    Softmax renormalization makes K and the final 1/sum exact:
        out = q / sum_valid(q)   on the valid [L,L] block
    and the host zero-fills the masked region of the output.
  * Engine placement per [128, 2048] row-block:
      PE : 4 bf16 matmuls (512-col PSUM banks)        ~1.9us
      ACT: p = Exp(scale*psum), fp16 out              ~2.0us
      DVE: t = (p + a) * p, accum -> qs   (stt is 1x on TRN2 HW: 2.27us;
           TENSOR_SCALAR is the only 4x op, and needs one tensor input)
      Pool: qs2 = qs + b*L (tiny) ; DVE: r = 1/qs2 (tiny)
      scale pass out = (t + b) * r: DVE tensor_scalar (4x, 0.75us) for
           19/32 blocks, ACT Identity(t*r + b*r) for 13/32 blocks —
           balances ACT ~90us vs DVE ~90us instead of DVE-bound 98us
      DMA: out row-block, alternating SWDGE (gpsimd) / HWDGE (sync)
  * Measured per-core: span ~113.5us, scalar/vector ~90us each,
    tensor ~61us, dma ~53us.  (baseline 144.6us; rel err 1.04e-2)
"""

import sys

import numpy as np

for _p in ("/opt/trn_rl_repo", "/root/.axon_site/_ro/trn_rl_repo"):
    if _p not in sys.path:
        sys.path.append(_p)

B, N, D = 16, 2048, 3
NCORES = 8
SPC = B // NCORES  # samples per core
P = 128            # SBUF partitions
MMF = 512          # matmul moving free-dim chunk (= 1 PSUM bank of fp32)
NB = SPC * N // P  # row-blocks per core
KAUG = 22          # 20 hi/lo aug rows + 2 mask-fold rows
MASKC = 144.0      # mask fold offset: masked entries get y - 144

MODE = "quad"      # "quad" (2 DVE passes) or "cubic" (3 DVE passes)
POOL_STT = 0       # Pool Q7 firmware lacks stt/ts; keep all stt on DVE
POOL_ACCUM = True  # Pool stt supports accum_out (fallback: extra DVE reduce)
ACT_SCALE = 13     # final-scale passes moved to ACT Copy (out = t*r + b*r)

# minimax fits of (poly in p) / (K * exp(exp(y))) - 1 over y <= 0
QS_S = 0.9943403856229558   # p = exp(QS_S * y)
QS_A = 1.05888673672267     # q = p^2 + QS_A*p + QS_B
QS_B = 1.217950642291432
CU_A = 1.600139700859946    # q = p^3 + CU_A*p^2 + CU_B*p + CU_C, p = exp(y)
CU_B = 3.7300379796011542
CU_C = 3.6840145818755072

_CACHE: dict = {}


def _build(mode):
    import concourse.bacc as bacc
    import concourse.tile as tile
    from concourse import mybir

    f32 = mybir.dt.float32
    f16 = mybir.dt.float16
    bf16 = mybir.dt.bfloat16
    AT = mybir.AluOpType
    nc = bacc.Bacc(None, target_bir_lowering=False, debug=False)

    aug_x = nc.dram_tensor("aug_x", [SPC, KAUG, N], bf16, kind="ExternalInput")
    aug_y = nc.dram_tensor("aug_y", [SPC, KAUG, N], bf16, kind="ExternalInput")
    # per-sample row-sum correction, replicated across partitions:
    # quad: b*L ; cubic: c*L
    cons = nc.dram_tensor("cons", [SPC, P, 1], f32, kind="ExternalInput")
    out = nc.dram_tensor("out", [SPC, N, N], f16, kind="ExternalOutput")

    o_flat = out.rearrange("s n m -> (s n) m")

    with tile.TileContext(nc) as tc:
        with (
            tc.tile_pool(name="consts", bufs=1) as consts,
            tc.tile_pool(name="work", bufs=10) as workp,
            tc.tile_pool(name="ot", bufs=8) as otp,
            tc.tile_pool(name="small", bufs=16) as smallp,
            tc.tile_pool(name="psum", bufs=2, space="PSUM") as psump,
        ):
            augx_t, augy_t, cons_t = [], [], []
            for s in range(SPC):
                ax = consts.tile([KAUG, N], bf16, tag=f"augx{s}")
                ay = consts.tile([KAUG, N], bf16, tag=f"augy{s}")
                cn = consts.tile([P, 1], f32, tag=f"cons{s}")
                nc.sync.dma_start(out=ax, in_=aug_x[s])
                nc.gpsimd.dma_start(out=ay, in_=aug_y[s])
                nc.sync.dma_start(out=cn, in_=cons[s])
                augx_t.append(ax)
                augy_t.append(ay)
                cons_t.append(cn)

            npool = 0
            for ib in range(NB):
                s = ib // (N // P)
                i0 = (ib % (N // P)) * P

                ps = psump.tile([P, N], f32)
                for j in range(N // MMF):
                    nc.tensor.matmul(
                        ps[:, j * MMF:(j + 1) * MMF],
                        augx_t[s][:, i0:i0 + P],
                        augy_t[s][:, j * MMF:(j + 1) * MMF],
                    )

                # Bresenham spread of POOL_STT pool-blocks over NB
                use_pool = ((ib + 1) * POOL_STT // NB) != (ib * POOL_STT // NB)
                npool += use_pool
                stt_eng = nc.gpsimd if use_pool else nc.vector

                p = workp.tile([P, N], f16, tag="p")
                qs = smallp.tile([P, 1], f32, tag="qs")
                if mode == "quad":
                    # p = exp(s*y); t = (p + a)*p ; qs = sum_j t
                    nc.scalar.activation(
                        p, ps, mybir.ActivationFunctionType.Exp, scale=QS_S
                    )
                    t = workp.tile([P, N], f16, tag="t")
                    if use_pool and not POOL_ACCUM:
                        stt_eng.scalar_tensor_tensor(
                            out=t, in0=p, scalar=QS_A, in1=p,
                            op0=AT.add, op1=AT.mult,
                        )
                        # row sum via a cheap DVE 4x pass into a scratch tile
                        tsc = workp.tile([P, N], f16, tag="tsc")
                        nc.vector.tensor_scalar(
                            out=tsc, in0=t, scalar1=1.0, scalar2=None,
                            op0=AT.mult, accum_out=qs,
                        )
                    else:
                        stt_eng.scalar_tensor_tensor(
                            out=t, in0=p, scalar=QS_A, in1=p,
                            op0=AT.add, op1=AT.mult, accum_out=qs,
                        )
                    cfin = QS_B
                else:
                    # p = exp(y); t1 = (p + a)*p ; t = (t1 + b)*p ; qs = sum t
                    nc.scalar.activation(p, ps, mybir.ActivationFunctionType.Exp)
                    t1 = workp.tile([P, N], f16, tag="t1")
                    stt_eng.scalar_tensor_tensor(
                        out=t1, in0=p, scalar=CU_A, in1=p,
                        op0=AT.add, op1=AT.mult,
                    )
                    t = workp.tile([P, N], f16, tag="t")
                    stt_eng.scalar_tensor_tensor(
                        out=t, in0=t1, scalar=CU_B, in1=p,
                        op0=AT.add, op1=AT.mult, accum_out=qs,
                    )
                    cfin = CU_C

                # qs2 = qs + const*L ; r = 1/qs2   (tiny [P,1] ops on DVE)
                qs2 = smallp.tile([P, 1], f32, tag="qs2")
                nc.gpsimd.tensor_tensor(
                    out=qs2, in0=qs, in1=cons_t[s], op=AT.add
                )
                r = smallp.tile([P, 1], f32, tag="r")
                nc.vector.reciprocal(r, qs2)

                # out = (t + cfin) * r
                ot = otp.tile([P, N], f16, tag="ot")
                use_act = ((ib + 1) * ACT_SCALE // NB) != (ib * ACT_SCALE // NB)
                if use_act:
                    # ACT Copy computes t*scale + bias with [P,1] APs
                    br = smallp.tile([P, 1], f32, tag="br")
                    nc.vector.tensor_scalar_mul(br, r, cfin)
                    nc.scalar.activation(
                        ot, t, mybir.ActivationFunctionType.Identity,
                        scale=r, bias=br,
                    )
                else:
                    nc.vector.tensor_scalar(
                        out=ot, in0=t, scalar1=cfin, scalar2=r,
                        op0=AT.add, op1=AT.mult,
                    )
                out_eng = nc.gpsimd if ib % 2 == 0 else nc.sync
                nc_eng = out_eng
                nc_eng.dma_start(out=o_flat[ib * P:(ib + 1) * P, :], in_=ot)

    nc.compile()
    return nc


def _lengths_from_masks(masks):
    """Per-sample valid lengths; verifies the product-prefix structure."""
    diag = np.einsum('bii->bi', masks)
    valid = (diag > 0.5).astype(np.float32)
    lengths = valid.sum(axis=1).astype(np.int64)
    # prefix check + product check (cheap, exact)
    n = masks.shape[1]
    pref = (np.arange(n)[None, :] < lengths[:, None]).astype(np.float32)
    if not np.array_equal(valid, pref):
        return None
    if not np.array_equal(masks, valid[:, :, None] * valid[:, None, :]):
        return None
    return lengths, valid


def _prepare(coordinates, masks, sigma):
    """Host-side prep: shard over cores, build augmented coordinates."""
    import ml_dtypes

    bf = ml_dtypes.bfloat16
    coords = np.ascontiguousarray(np.asarray(coordinates, dtype=np.float32))
    masks = np.asarray(masks, dtype=np.float32)
    sig = float(np.asarray(sigma, dtype=np.float32).reshape(-1)[0])

    res = _lengths_from_masks(masks)
    assert res is not None, "masks are not product-of-prefix form"
    lengths, valid = res

    norms = np.sum(coords * coords, axis=2, dtype=np.float32)  # [B, N]
    xT = np.swapaxes(coords, 1, 2)                             # [B, 3, N]
    nss = np.float32(-1.0 / (sig * sig))
    aug_x = np.empty((B, 5, N), np.float32)
    aug_x[:, 0:3] = (-2.0 * nss) * xT
    aug_x[:, 3] = nss * norms
    aug_x[:, 4] = nss
    aug_y = np.empty((B, 5, N), np.float32)
    aug_y[:, 0:3] = xT
    aug_y[:, 3] = 1.0
    aug_y[:, 4] = norms

    # hi/lo bf16 split: v = hi + lo, K=5 fp32 -> K=20 bf16 contraction
    xh = aug_x.astype(bf)
    xl = (aug_x - xh.astype(np.float32)).astype(bf)
    yh = aug_y.astype(bf)
    yl = (aug_y - yh.astype(np.float32)).astype(bf)
    # mask fold rows: C*v_i*v_j - C  (exact in bf16: C=144, v in {0,1})
    C = np.float32(MASKC)
    mx = np.stack([C * valid, np.full_like(valid, C)], axis=1).astype(bf)
    my = np.stack([valid, np.full_like(valid, -1.0)], axis=1).astype(bf)
    augx22 = np.concatenate([xh, xl, xh, xl, mx], axis=1)  # [B, 22, N]
    augy22 = np.concatenate([yh, yh, yl, yl, my], axis=1)

    ccoef = QS_B if MODE == "quad" else CU_C
    consv = (np.float32(ccoef) * lengths.astype(np.float32))  # [B]
    cons = np.broadcast_to(consv[:, None, None], (B, P, 1)).astype(np.float32)

    in_maps = []
    for c in range(NCORES):
        lo, hi = c * SPC, (c + 1) * SPC
        in_maps.append({
            "aug_x": np.ascontiguousarray(augx22[lo:hi]),
            "aug_y": np.ascontiguousarray(augy22[lo:hi]),
            "cons": np.ascontiguousarray(cons[lo:hi]),
        })
    return in_maps, lengths


def _get_nc():
    if "nc" not in _CACHE:
        _CACHE["nc"] = _build(MODE)
    return _CACHE["nc"]


def kernel(coordinates, masks, sigma):
    import time

    from concourse.bass_utils import run_bass_kernel_spmd

    in_maps, lengths = _prepare(coordinates, masks, sigma)
    # the shared trn2 device occasionally reports a transient
    # NRT_EXEC_UNIT_UNRECOVERABLE; it clears on its own within ~a minute
    for attempt in range(4):
        try:
            res = run_bass_kernel_spmd(
                _get_nc(), in_maps, core_ids=list(range(NCORES))
            )
            break
        except Exception:  # noqa: BLE001 - retry transient device errors
            if attempt == 3:
                raise
            time.sleep(20 * (attempt + 1))

    full = np.zeros((B, N, N), np.float32)
    for b in range(B):
        c, s = b // SPC, b % SPC
        L = int(lengths[b])
        full[b, :L, :L] = res.results[c]["out"][s, :L, :L].astype(np.float32)
    return full
